# revision 45
# baseline (speedup 1.0000x reference)
"""CrossCovarianceAttn Trainium2 kernel.

Data-parallel over B=8 across 8 NeuronCores; each core runs the full model on
one batch element.

Numerics: the q/k projection and the per-head covariance/Gram matmuls run in
fp8e4m3 with perf_mode=DoubleRow (two 128-row k-tiles per instruction, 0.5
cyc/row) — safe because q,k are l2-normalized over the token dim downstream,
which cancels the fp8 scaling exactly and the softmax logits are small
(|logit| <= temperature by Cauchy-Schwarz, which also lets softmax skip the
max-subtraction). w_q/w_k/w_v are scaled x64 so fp8/bf16 see them in their
normal range; v carries the x64 until the psum->sbuf cast divides it back
out. The v projection, attn@v and the output projection run in bf16 with the
projection contracting dense 128-row c-blocks.

x and the weights are loaded with GpSimd (SWDGE) cast-DMAs that convert
fp32->bf16 in flight (verified bit-exact on device), halving their DMA time;
all PE transposes then run on bf16 data with a bf16 identity (1.0 cyc/row).

Cross-partition relayouts use sbuf->sbuf strip DMAs (compute engines cannot
cross partitions): vT (c-major [128,6,N] from the PE) -> vt_all [96,H,N]
per-head layout for attn@v, and attn@v output (per-head [96,H,N]) ->
otsb [128,6,N] dense c-major for the projection. Both are batched over
2-tile pairs to halve the per-DMA HWDGE fixed cost.

Pipeline per core:
  phase 0: w_qkv -> w_qk_f8 (plain layout, 3 per-512-col tiles so the first
           qk matmul only waits on a third of the prep) + w_vT bf16 (x64),
           interleaved with the first two x stages
  phase 1: per 512-token tile: PE-transpose x -> xT psum; cast to xT_bf +
           xT_f8 (DVE/ACT); qk = xT_f8.T @ w_qk_f8 (DoubleRow) -> qk_t fp8;
           covariance+Gram (3 DoubleRow matmuls per head: Gq | Gk | C)
           accumulated over pairs of tiles into cg_accum fp32; vT = w_vT.T @
           xT_bf -> strip-DMA into vt_all (deferred for the last tile pair
           so it fills the PE during the phase-2 softmax chain)
  phase 2: fused Gram-diagonal extract -> 1/max(||.||,eps); batched all-head
           softmax (no max-sub); transpose attn -> attnT bf16
  phase 3 (sw-pipelined over 2-tile groups): attn_h @ vt_all_h -> ot96 ->
           strip-DMA -> otsb; y = otsb.T @ w_projT + b -> out
"""
import os
import sys

sys.path.insert(0, "/opt/trn_rl_repo")

import numpy as np

import concourse.bass as bass
import concourse.mybir as mybir
import concourse.tile as tile
from concourse import bacc
from concourse.bass_utils import run_bass_kernel_spmd
from concourse.masks import make_identity

FP32 = mybir.dt.float32
FP32R = mybir.dt.float32r
BF16 = mybir.dt.bfloat16
FP8 = mybir.dt.float8e4
DR = mybir.MatmulPerfMode.DoubleRow

N_TOK = 4096
C = 768
H = 8
HD = 96
C3 = 3 * C
TOK_TILE = 512
N_TILES = N_TOK // TOK_TILE
CHUNKS = TOK_TILE // 128
KK = C // 128
EPS = 1e-12

S_W = 64.0           # w_q/w_k (and w_v) -> fp8/bf16 scale
C_QK = 26.0 / 35.5   # qk psum (scaled x64) -> fp8 scale

_CACHED_NC = None
XTP_BUFS = int(os.environ.get("XTP_BUFS", "3"))
QKP_BUFS = int(os.environ.get("QKP_BUFS", "3"))
PSTR_BUFS = int(os.environ.get("PSTR_BUFS", "2"))
PSMM_BUFS = int(os.environ.get("PSMM_BUFS", "6"))
XIN_BUFS = int(os.environ.get("XIN_BUFS", "3"))


def _vt_strips():
    """Strips (m, p0, run, h, d0): vt_sb partition p of block m holds
    v-feature c = 128m + p = 96h + d -> vt_all partition d, head h."""
    strips = []
    for m in range(KK):
        c0 = 128 * m
        p = 0
        while p < 128:
            h, d = divmod(c0 + p, HD)
            run = min(128 - p, HD - d)
            strips.append((m, p, run, h, d))
            p += run
    return strips


def _ot_strips():
    """Strips (h, d0, run, m, p0): attnv psum for head h, row d holds
    out-feature c = 96h + d -> otsb128 partition p = c % 128, block m."""
    strips = []
    for h in range(H):
        c0 = HD * h
        d = 0
        while d < HD:
            m, p = divmod(c0 + d, 128)
            run = min(HD - d, 128 - p)
            strips.append((h, d, run, m, p))
            d += run
    return strips


def phase2(nc, tc, singles, dram, ps_tr, cg_accum, attnT, temp_all, ident96f):
    """Norms + batched all-head softmax -> attnT (bf16).

    cg_accum[:, h, :]: [0:96] Gq, [96:192] Gk, [192:288] C. The Gq|Gk
    adjacency lets one mask-mult + one reduce grab both sets of diagonals.
    |logit| <= temperature (Cauchy-Schwarz on unit vectors), so exp runs
    directly with no max-subtraction.
    """
    import concourse.mybir as mybir

    identb = ident96f[:, None, None, :].to_broadcast((HD, H, 2, HD))
    sq = singles.tile([HD, H, 2], FP32)
    scr = singles.tile([HD, H, 2, HD], FP32)
    nc.vector.tensor_tensor(
        scr, cg_accum[:, :, 0:2 * HD].rearrange(
            "d h (two e) -> d h two e", two=2),
        identb, mybir.AluOpType.mult)
    nc.vector.reduce_sum(sq[:, :, :, None], scr, axis=mybir.AxisListType.X)

    nrm = singles.tile([HD, H, 2], FP32)
    nc.scalar.sqrt(nrm, sq)
    nc.vector.tensor_scalar_max(nrm, nrm, EPS)
    rnorm = singles.tile([HD, H, 2], FP32)
    nc.vector.reciprocal(rnorm, nrm)
    rq = singles.tile([HD, H], FP32)
    nc.vector.tensor_tensor(rq, rnorm[:, :, 0], temp_all,
                            mybir.AluOpType.mult)

    # rk to the free dim: store h-major to DRAM, broadcast-read back
    rk_scr = dram.tile([H, HD], FP32)
    nc.sync.dma_start(
        bass.AP(tensor=rk_scr.tensor, offset=rk_scr.offset,
                ap=[[1, HD], [HD, H]]),
        rnorm[:, :, 1])
    rk_all = singles.tile([HD, H, HD], FP32)
    nc.sync.dma_start(
        rk_all, bass.AP(tensor=rk_scr.tensor, offset=rk_scr.offset,
                        ap=[[0, HD], [1, H * HD]]))

    attL = singles.tile([HD, H, HD], FP32)
    nc.vector.tensor_tensor(
        attL, cg_accum[:, :, 2 * HD:3 * HD],
        rq[:, :, None].to_broadcast((HD, H, HD)), mybir.AluOpType.mult)
    nc.vector.tensor_tensor(attL, attL, rk_all, mybir.AluOpType.mult)
    nc.scalar.activation(attL, attL, mybir.ActivationFunctionType.Exp)
    sea = singles.tile([HD, H, 1], FP32)
    nc.vector.reduce_sum(sea, attL, axis=mybir.AxisListType.X)
    rsea = singles.tile([HD, H, 1], FP32)
    nc.vector.reciprocal(rsea, sea)
    nc.vector.tensor_tensor(
        attL, attL, rsea.to_broadcast((HD, H, HD)), mybir.AluOpType.mult)
    for h in range(H):
        atps = ps_tr.tile([HD, HD], FP32, name="atps", tag="tr")
        nc.tensor.transpose(atps, attL[:, h, :], ident96f)
        if h % 2 == 0:
            nc.vector.tensor_copy(attnT[:, h, :], atps)
        else:
            nc.scalar.copy(attnT[:, h, :], atps)


def build_nc():
    nc = bacc.Bacc("TRN2", target_bir_lowering=False, debug=False, num_devices=8)

    x_d = nc.dram_tensor("x", (N_TOK, C), FP32R, kind="ExternalInput").ap()
    wqkv_d = nc.dram_tensor("w_qkv", (C3, C), FP32R, kind="ExternalInput").ap()
    temp_d = nc.dram_tensor("temperature", (H, 1, 1), FP32, kind="ExternalInput").ap()
    wproj_d = nc.dram_tensor("w_proj", (C, C), FP32R, kind="ExternalInput").ap()
    bproj_d = nc.dram_tensor("b_proj", (C,), FP32, kind="ExternalInput").ap()
    out_d = nc.dram_tensor("out", (N_TOK, C), FP32, kind="ExternalOutput").ap()

    with tile.TileContext(nc) as tc:
        _build(tc, nc, x_d, wqkv_d, temp_d, wproj_d, bproj_d, out_d)
    nc.compile()
    return nc


def _build(tc, nc, x_d, wqkv_d, temp_d, wproj_d, bproj_d, out_d):
    import contextlib

    ctx = contextlib.ExitStack()
    with ctx:
        singles = ctx.enter_context(tc.tile_pool(name="singles", bufs=1))
        dram = ctx.enter_context(tc.tile_pool(name="dram", bufs=1, space="DRAM"))
        ps_tr = ctx.enter_context(tc.tile_pool(name="ps_tr", bufs=PSTR_BUFS, space="PSUM"))

        ident_f32 = singles.tile([128, 128], FP32)
        make_identity(nc, ident_f32)
        ident = singles.tile([128, 128], BF16)
        nc.vector.tensor_copy(ident, ident_f32)
        ident96f = ident_f32[0:HD, 0:HD]

        b_all = singles.tile([128, C], FP32)
        nc.gpsimd.dma_start(
            b_all, bass.AP(tensor=bproj_d.tensor, offset=bproj_d.offset,
                           ap=[[0, 128], [1, C]]))
        temp_all = singles.tile([HD, H], FP32)
        nc.gpsimd.dma_start(
            temp_all, bass.AP(tensor=temp_d.tensor, offset=temp_d.offset,
                              ap=[[0, HD], [1, H]]))

        warm = singles.tile([1, 1], FP32)
        nc.vector.memset(warm, 0.5)
        nc.scalar.activation(warm, warm, mybir.ActivationFunctionType.Exp)
        nc.scalar.sqrt(warm, warm)

        cg_accum = singles.tile([HD, H, 288], FP32)
        nc.vector.memset(cg_accum, 0.0)
        attnT = singles.tile([HD, H, HD], BF16)
        vt_all = singles.tile([HD, H, N_TOK], BF16)

        # ---------------- phase 0 + 1 ----------------
        # w_qk plain layout: w_qk_f8[p][:, kk, j] = 64 * w_qkv[512p+j, 128kk+:]
        # (cols 0..767 across the 3 tiles = q rows, 768..1535 = k rows)
        with tc.tile_pool(name="wqk_pool", bufs=1) as wqk_pool, \
             tc.tile_pool(name="xin", bufs=XIN_BUFS) as xin, \
             tc.tile_pool(name="xtp", bufs=XTP_BUFS) as xtp, \
             tc.tile_pool(name="qkp", bufs=QKP_BUFS) as qkp, \
             tc.tile_pool(name="vtsb", bufs=2) as vtsb, \
             tc.tile_pool(name="ps_mm", bufs=PSMM_BUFS, space="PSUM") as ps_mm:
            w_qk_f8 = [wqk_pool.tile([128, KK, 512], FP8, name=f"wqk{p}")
                       for p in range(3)]
            w_vT = wqk_pool.tile([128, KK, C], BF16)   # holds 64*w_v
            vstrips = _vt_strips()
            state = {"vt_sb": None, "qk_pair": []}

            def xstage(t):
                """x load + bf16 pre-cast + PE transpose + bf16/fp8 casts."""
                t0 = t * TOK_TILE
                x_t = xin.tile([128, CHUNKS, C], BF16, name="x_t")
                nc.gpsimd.dma_start(
                    x_t, x_d[t0:t0 + TOK_TILE, :].rearrange(
                        "(c p) f -> p c f", p=128))
                xT_bf = xtp.tile([128, KK, TOK_TILE], BF16, name="xT_bf")
                xT_f8 = xtp.tile([128, KK, TOK_TILE], FP8, name="xT_f8")
                for kk in range(KK):
                    xps = ps_tr.tile([128, TOK_TILE], BF16, name="xps",
                                     tag="tr")
                    for c in range(CHUNKS):
                        nc.tensor.transpose(
                            xps[:, c * 128:(c + 1) * 128],
                            x_t[:, c, kk * 128:(kk + 1) * 128], ident)
                    if kk % 2 == 0:
                        nc.vector.tensor_copy(xT_bf[:, kk, :], xps)
                        nc.scalar.copy(xT_f8[:, kk, :], xps)
                    else:
                        nc.scalar.copy(xT_bf[:, kk, :], xps)
                        nc.vector.tensor_copy(xT_f8[:, kk, :], xps)
                return xT_bf, xT_f8

            def mmstage(t, xT_bf, xT_f8):
                """qk + vT matmuls, vt strips, covariance for one tile."""
                t0 = t * TOK_TILE
                qk_t = qkp.tile([128, CHUNKS, 1536], FP8, name="qk_t")

                # qk = xT.T @ w_qk (token-major; fp8 DoubleRow pairs)
                for c in range(CHUNKS):
                    for p in range(3):
                        mmps = ps_mm.tile([128, 512], FP32, name="mmps",
                                          tag="s")
                        for i in range(KK // 2):
                            nc.tensor.matmul(
                                mmps,
                                xT_f8[:, 2 * i:2 * i + 2,
                                      c * 128:(c + 1) * 128],
                                w_qk_f8[p][:, 2 * i:2 * i + 2, :],
                                start=(i == 0), stop=(i == KK // 2 - 1),
                                perf_mode=DR)
                        if (c * 3 + p) % 2 == 0:
                            nc.scalar.mul(
                                qk_t[:, c, p * 512:(p + 1) * 512], mmps, C_QK)
                        else:
                            nc.vector.tensor_scalar_mul(
                                qk_t[:, c, p * 512:(p + 1) * 512], mmps, C_QK)

                # covariance + Gram over a pair of tiles: per head
                # [Gq | Gk | C] = [q'q | k'k | q'k], DoubleRow chunk pairs.
                # Emitted before vT on odd tiles so the final flush (and the
                # phase-2 chain it gates) overlaps the last tile's vT matmuls.
                state["qk_pair"].append(qk_t)
                if t % 2 == 1:
                    qk_pair = state["qk_pair"]
                    for h in range(H):
                        cg_ps = ps_mm.tile([HD, 288], FP32, name="cg_ps",
                                           tag="s")
                        np_ = 2 * len(qk_pair)
                        for i in range(np_):
                            qkx = qk_pair[i // 2]
                            lo = (i % 2) * 2
                            q_sl = qkx[:, lo:lo + 2, HD * h:HD * h + HD]
                            k_sl = qkx[:, lo:lo + 2,
                                       C + HD * h:C + HD * h + HD]
                            nc.tensor.matmul(
                                cg_ps[:, 0:HD], q_sl, q_sl,
                                start=(i == 0), stop=False, perf_mode=DR)
                            nc.tensor.matmul(
                                cg_ps[:, HD:2 * HD], k_sl, k_sl,
                                start=False, stop=False, perf_mode=DR)
                            nc.tensor.matmul(
                                cg_ps[:, 2 * HD:3 * HD], q_sl, k_sl,
                                start=False, stop=(i == np_ - 1),
                                perf_mode=DR)
                        nc.vector.tensor_add(
                            cg_accum[:, h, :], cg_ps, cg_accum[:, h, :])
                    state["qk_pair"] = []

            def vtstage(t, xT_bf, act_only=False):
                # vT = w_vT.T @ xT_bf (feature-major, c-major blocks);
                # batched over pairs of tiles to halve strip-DMA count
                if t % 2 == 0:
                    state["vt_sb"] = vtsb.tile([128, KK, 2 * TOK_TILE], BF16,
                                               name="vt_sb")
                vt_sb = state["vt_sb"]
                half = (t % 2) * TOK_TILE
                for m in range(KK):
                    vps = ps_mm.tile([128, TOK_TILE], FP32, name="vps",
                                     tag="s")
                    for kk in range(KK):
                        nc.tensor.matmul(
                            vps, w_vT[:, kk, m * 128:(m + 1) * 128],
                            xT_bf[:, kk, :],
                            start=(kk == 0), stop=(kk == KK - 1))
                    if act_only or m % 2 == 0:
                        nc.scalar.mul(
                            vt_sb[:, m, half:half + TOK_TILE], vps, 1.0 / S_W)
                    else:
                        nc.vector.tensor_scalar_mul(
                            vt_sb[:, m, half:half + TOK_TILE], vps, 1.0 / S_W)
                if t % 2 == 1:
                    tp0 = (t - 1) * TOK_TILE
                    for si, (m, p0, run, h, d0) in enumerate(vstrips):
                        src = vt_sb[p0:p0 + run, m, :]
                        dst = vt_all[d0:d0 + run, h, tp0:tp0 + 2 * TOK_TILE]
                        if si % 3 == 0:
                            nc.sync.dma_start(dst, src)
                        elif si % 3 == 1:
                            nc.scalar.dma_start(dst, src)
                        else:
                            nc.gpsimd.dma_start(dst, src)

            # w prep in groups of 4 row-blocks: one wide psum + one cast per
            # (group, kk) instead of 4 narrow ones, using the ps_mm banks
            # that sit idle until the first qk matmul. Interleaved with the
            # first two x stages: the first qk matmul only needs w_qk
            # p-block 0 (m 0..3) + xT(0).
            xT01 = [None, None]
            with tc.tile_pool(name="wload", bufs=2) as wload:
                def wprep(grp):
                    w_blk = wload.tile([128, 2, C], BF16, name="w_blk")
                    nc.gpsimd.dma_start(
                        w_blk,
                        wqkv_d[grp * 256:(grp + 1) * 256, :].rearrange(
                            "(b p) f -> p b f", p=128))
                    for kk in range(KK):
                        tps = ps_mm.tile([128, 256], BF16, name="wps",
                                         tag="s")
                        for b in range(2):
                            nc.tensor.transpose(
                                tps[:, b * 128:(b + 1) * 128],
                                w_blk[:, b, kk * 128:(kk + 1) * 128], ident)
                        if grp < 6:
                            dst = w_qk_f8[grp // 2][
                                :, kk, (grp % 2) * 256:(grp % 2) * 256 + 256]
                            if (grp + kk) % 2 == 0:
                                nc.vector.tensor_scalar_mul(dst, tps, S_W)
                            else:
                                nc.scalar.mul(dst, tps, S_W)
                        else:
                            base = (grp - 6) * 256
                            dst = w_vT[:, kk, base:base + 256]
                            if kk % 2 == 0:
                                nc.vector.tensor_scalar_mul(dst, tps, S_W)
                            else:
                                nc.scalar.mul(dst, tps, S_W)

                xT01[0] = xstage(0)
                wprep(0)
                wprep(1)
                xT01[1] = xstage(1)
                wprep(2)
                wprep(3)
                for grp in range(4, 9):
                    wprep(grp)

            for t in range(N_TILES):
                xts = xT01[t] if t < 2 else state.pop(("x", t))
                mmstage(t, *xts)
                if t < 6:
                    vtstage(t, xts[0])
                else:
                    state[("xts", t)] = xts
                if t + 2 < N_TILES:
                    state[("x", t + 2)] = xstage(t + 2)

            phase2(nc, tc, singles, dram, ps_tr, cg_accum, attnT, temp_all,
                   ident96f)

            # deferred vT for the last tile pair: fills the PE while the
            # phase-2 DVE/ACT softmax chain runs
            vtstage(6, state.pop(("xts", 6))[0])
            vtstage(7, state.pop(("xts", 7))[0])

        # ---------------- phase 3 pools; w_projT prep emitted first so the
        # PE has work while the DVE/ACT-heavy phase 2 chain runs ----------
        with tc.tile_pool(name="wpp", bufs=1) as wpp, \
             tc.tile_pool(name="wpload", bufs=2) as wpload, \
             tc.tile_pool(name="ot96p", bufs=2) as ot96p, \
             tc.tile_pool(name="otp", bufs=2) as otp, \
             tc.tile_pool(name="yp", bufs=2) as yp, \
             tc.tile_pool(name="ps_o", bufs=3, space="PSUM") as ps_o, \
             tc.tile_pool(name="ps_y", bufs=3, space="PSUM") as ps_y:
            # w_proj (cout, c) -> w_projT128 [128, m, cout] (dense c-major)
            w_projT = wpp.tile([128, KK, C], BF16)
            for n in range(KK):
                wp_blk = wpload.tile([128, C], BF16, name="wp_blk")
                nc.gpsimd.dma_start(wp_blk, wproj_d[n * 128:(n + 1) * 128, :])
                for m in range(KK):
                    tps2 = ps_tr.tile([128, 128], BF16, name="tps2", tag="tr")
                    nc.tensor.transpose(
                        tps2, wp_blk[:, m * 128:(m + 1) * 128], ident)
                    if (n + m) % 2 == 0:
                        nc.vector.tensor_copy(
                            w_projT[:, m, n * 128:(n + 1) * 128], tps2)
                    else:
                        nc.scalar.copy(
                            w_projT[:, m, n * 128:(n + 1) * 128], tps2)

            # ---------------- phase 3: attn@v + proj, sw-pipelined over
            # 2-tile groups (halves the relayout strip-DMA count) ----------
            ostrips = _ot_strips()
            T2 = 2 * TOK_TILE

            def attnv_group(g):
                g0 = g * T2
                ot96 = ot96p.tile([HD, H, T2], BF16, name="ot96")
                otsb = otp.tile([128, KK, T2], BF16, name="otsb")
                for half in range(2):
                    t0 = g0 + half * TOK_TILE
                    for h in range(H):
                        ops_ = ps_o.tile([HD, TOK_TILE], FP32, name="ops_")
                        nc.tensor.matmul(ops_, attnT[:, h, :],
                                         vt_all[:, h, t0:t0 + TOK_TILE],
                                         start=True, stop=True)
                        hh0 = half * TOK_TILE
                        if h % 2 == 0:
                            nc.vector.tensor_copy(
                                ot96[:, h, hh0:hh0 + TOK_TILE], ops_)
                        else:
                            nc.scalar.copy(
                                ot96[:, h, hh0:hh0 + TOK_TILE], ops_)
                for si, (h, d0, run, m, p0) in enumerate(ostrips):
                    src = ot96[d0:d0 + run, h, :]
                    dst = otsb[p0:p0 + run, m, :]
                    if si % 3 == 0:
                        nc.sync.dma_start(dst, src)
                    elif si % 3 == 1:
                        nc.scalar.dma_start(dst, src)
                    else:
                        nc.gpsimd.dma_start(dst, src)
                return otsb

            def proj_group(g, otsb):
                for piece in range(CHUNKS):
                    t0 = g * T2 + piece * 256
                    y_t = yp.tile([128, 2, C], FP32, name="y_t")
                    for c in range(2):
                        cc = piece * 2 + c
                        for (off, width) in ((0, 512), (512, 256)):
                            yps = ps_y.tile([128, 512], FP32, name="yps")
                            for m in range(KK):
                                nc.tensor.matmul(
                                    yps[:, :width],
                                    otsb[:, m, cc * 128:(cc + 1) * 128],
                                    w_projT[:, m, off:off + width],
                                    start=(m == 0), stop=(m == KK - 1))
                            nc.vector.tensor_tensor(
                                y_t[:, c, off:off + width], yps[:, :width],
                                b_all[:, off:off + width], mybir.AluOpType.add)
                    nc.scalar.dma_start(
                        out_d[t0:t0 + 256, :].rearrange(
                            "(c p) f -> p c f", p=128),
                        y_t)

            pend = None
            for g in range(N_TILES // 2):
                cur = attnv_group(g)
                if pend is not None:
                    proj_group(*pend)
                pend = (g, cur)
            proj_group(*pend)


def _get_nc():
    global _CACHED_NC
    if _CACHED_NC is None:
        _CACHED_NC = build_nc()
    return _CACHED_NC


def kernel(x, w_qkv, temperature, w_proj, b_proj):
    nc = _get_nc()
    x = np.ascontiguousarray(np.asarray(x, dtype=np.float32))
    in_maps = []
    for b in range(8):
        in_maps.append({
            "x": x[b],
            "w_qkv": np.asarray(w_qkv, dtype=np.float32),
            "temperature": np.asarray(temperature, dtype=np.float32),
            "w_proj": np.asarray(w_proj, dtype=np.float32),
            "b_proj": np.asarray(b_proj, dtype=np.float32),
        })
    res = run_bass_kernel_spmd(nc, in_maps, core_ids=list(range(8)))
    return np.stack([r["out"] for r in res.results], axis=0)


# revision 52
# speedup vs baseline: 1.0088x; 1.0088x over previous
"""CrossCovarianceAttn Trainium2 kernel.

Data-parallel over B=8 across 8 NeuronCores; each core runs the full model on
one batch element.

Numerics: the q/k projection and the per-head covariance/Gram matmuls run in
fp8e4m3 with perf_mode=DoubleRow (two 128-row k-tiles per instruction, 0.5
cyc/row) — safe because q,k are l2-normalized over the token dim downstream,
which cancels the fp8 scaling exactly and the softmax logits are small
(|logit| <= temperature by Cauchy-Schwarz, which also lets softmax skip the
max-subtraction). w_q/w_k/w_v are scaled x64 so fp8/bf16 see them in their
normal range; v carries the x64 until the psum->sbuf cast divides it back
out. The v projection, attn@v and the output projection run in bf16 with the
projection contracting dense 128-row c-blocks.

x and the weights are loaded with GpSimd (SWDGE) cast-DMAs that convert
fp32->bf16 in flight (verified bit-exact on device), halving their DMA time;
all PE transposes then run on bf16 data with a bf16 identity (1.0 cyc/row).

Cross-partition relayouts use sbuf->sbuf strip DMAs (compute engines cannot
cross partitions): vT (c-major [128,6,N] from the PE) -> vt_all [96,H,N]
per-head layout for attn@v, and attn@v output (per-head [96,H,N]) ->
otsb [128,6,N] dense c-major for the projection. Both are batched over
2-tile pairs to halve the per-DMA HWDGE fixed cost.

Pipeline per core:
  phase 0: w_qkv -> w_qk_f8 (plain layout, 3 per-512-col tiles so the first
           qk matmul only waits on a third of the prep) + w_vT bf16 (x64),
           interleaved with the first two x stages
  phase 1: per 512-token tile: PE-transpose x -> xT psum; cast to xT_bf +
           xT_f8 (DVE/ACT); qk = xT_f8.T @ w_qk_f8 (DoubleRow) -> qk_t fp8;
           covariance+Gram (3 DoubleRow matmuls per head: Gq | Gk | C)
           accumulated over pairs of tiles into cg_accum fp32; vT = w_vT.T @
           xT_bf -> strip-DMA into vt_all (deferred for the last tile pair
           so it fills the PE during the phase-2 softmax chain)
  phase 2: fused Gram-diagonal extract -> 1/max(||.||,eps); batched all-head
           softmax (no max-sub); transpose attn -> attnT bf16
  phase 3 (sw-pipelined over 2-tile groups): attn_h @ vt_all_h -> ot96 ->
           strip-DMA -> otsb; y = otsb.T @ w_projT + b -> out
"""
import os
import sys

sys.path.insert(0, "/opt/trn_rl_repo")

import numpy as np

import concourse.bass as bass
import concourse.mybir as mybir
import concourse.tile as tile
from concourse import bacc
from concourse.bass_utils import run_bass_kernel_spmd
from concourse.masks import make_identity

FP32 = mybir.dt.float32
FP32R = mybir.dt.float32r
BF16 = mybir.dt.bfloat16
FP8 = mybir.dt.float8e4
DR = mybir.MatmulPerfMode.DoubleRow

N_TOK = 4096
C = 768
H = 8
HD = 96
C3 = 3 * C
TOK_TILE = 512
N_TILES = N_TOK // TOK_TILE
CHUNKS = TOK_TILE // 128
KK = C // 128
EPS = 1e-12

S_W = 64.0           # w_q/w_k (and w_v) -> fp8/bf16 scale
C_QK = 26.0 / 35.5   # qk psum (scaled x64) -> fp8 scale

_CACHED_NC = None
XTP_BUFS = int(os.environ.get("XTP_BUFS", "3"))
QKP_BUFS = int(os.environ.get("QKP_BUFS", "3"))
PSTR_BUFS = int(os.environ.get("PSTR_BUFS", "2"))
PSMM_BUFS = int(os.environ.get("PSMM_BUFS", "6"))
XIN_BUFS = int(os.environ.get("XIN_BUFS", "3"))


def _vt_strips():
    """Strips (m, p0, run, h, d0): vt_sb partition p of block m holds
    v-feature c = 128m + p = 96h + d -> vt_all partition d, head h."""
    strips = []
    for m in range(KK):
        c0 = 128 * m
        p = 0
        while p < 128:
            h, d = divmod(c0 + p, HD)
            run = min(128 - p, HD - d)
            strips.append((m, p, run, h, d))
            p += run
    return strips


def _ot_strips():
    """Strips (h, d0, run, m, p0): attnv psum for head h, row d holds
    out-feature c = 96h + d -> otsb128 partition p = c % 128, block m."""
    strips = []
    for h in range(H):
        c0 = HD * h
        d = 0
        while d < HD:
            m, p = divmod(c0 + d, 128)
            run = min(HD - d, 128 - p)
            strips.append((h, d, run, m, p))
            d += run
    return strips


def phase2(nc, tc, singles, dram, ps_tr, cg_accum, attnT, temp_all, ident96f):
    """Norms + batched all-head softmax -> attnT (bf16).

    cg_accum[:, h, :]: [0:96] Gq, [96:192] Gk, [192:288] C. The Gq|Gk
    adjacency lets one mask-mult + one reduce grab both sets of diagonals.
    |logit| <= temperature (Cauchy-Schwarz on unit vectors), so exp runs
    directly with no max-subtraction.
    """
    import concourse.mybir as mybir

    identb = ident96f[:, None, None, :].to_broadcast((HD, H, 2, HD))
    sq = singles.tile([HD, H, 2], FP32)
    scr = singles.tile([HD, H, 2, HD], FP32)
    nc.vector.tensor_tensor(
        scr, cg_accum[:, :, 0:2 * HD].rearrange(
            "d h (two e) -> d h two e", two=2),
        identb, mybir.AluOpType.mult)
    nc.vector.reduce_sum(sq[:, :, :, None], scr, axis=mybir.AxisListType.X)

    nrm = singles.tile([HD, H, 2], FP32)
    nc.scalar.sqrt(nrm, sq)
    nc.vector.tensor_scalar_max(nrm, nrm, EPS)
    rnorm = singles.tile([HD, H, 2], FP32)
    nc.vector.reciprocal(rnorm, nrm)
    rq = singles.tile([HD, H], FP32)
    nc.vector.tensor_tensor(rq, rnorm[:, :, 0], temp_all,
                            mybir.AluOpType.mult)

    # rk to the free dim: store h-major to DRAM, broadcast-read back
    rk_scr = dram.tile([H, HD], FP32)
    nc.sync.dma_start(
        bass.AP(tensor=rk_scr.tensor, offset=rk_scr.offset,
                ap=[[1, HD], [HD, H]]),
        rnorm[:, :, 1])
    rk_all = singles.tile([HD, H, HD], FP32)
    nc.sync.dma_start(
        rk_all, bass.AP(tensor=rk_scr.tensor, offset=rk_scr.offset,
                        ap=[[0, HD], [1, H * HD]]))

    attL = singles.tile([HD, H, HD], FP32)
    nc.vector.tensor_tensor(
        attL, cg_accum[:, :, 2 * HD:3 * HD],
        rq[:, :, None].to_broadcast((HD, H, HD)), mybir.AluOpType.mult)
    nc.vector.tensor_tensor(attL, attL, rk_all, mybir.AluOpType.mult)
    nc.scalar.activation(attL, attL, mybir.ActivationFunctionType.Exp)
    sea = singles.tile([HD, H, 1], FP32)
    nc.vector.reduce_sum(sea, attL, axis=mybir.AxisListType.X)
    rsea = singles.tile([HD, H, 1], FP32)
    nc.vector.reciprocal(rsea, sea)
    nc.vector.tensor_tensor(
        attL, attL, rsea.to_broadcast((HD, H, HD)), mybir.AluOpType.mult)
    for h in range(H):
        atps = ps_tr.tile([HD, HD], FP32, name="atps", tag="tr")
        nc.tensor.transpose(atps, attL[:, h, :], ident96f)
        if h % 2 == 0:
            nc.vector.tensor_copy(attnT[:, h, :], atps)
        else:
            nc.scalar.copy(attnT[:, h, :], atps)


def build_nc():
    nc = bacc.Bacc("TRN2", target_bir_lowering=False, debug=False, num_devices=8)

    x_d = nc.dram_tensor("x", (N_TOK, C), FP32R, kind="ExternalInput").ap()
    wqkv_d = nc.dram_tensor("w_qkv", (C3, C), FP32R, kind="ExternalInput").ap()
    temp_d = nc.dram_tensor("temperature", (H, 1, 1), FP32, kind="ExternalInput").ap()
    wproj_d = nc.dram_tensor("w_proj", (C, C), FP32R, kind="ExternalInput").ap()
    bproj_d = nc.dram_tensor("b_proj", (C,), FP32, kind="ExternalInput").ap()
    out_d = nc.dram_tensor("out", (N_TOK, C), FP32, kind="ExternalOutput").ap()

    with tile.TileContext(nc) as tc:
        _build(tc, nc, x_d, wqkv_d, temp_d, wproj_d, bproj_d, out_d)
    nc.compile()
    return nc


def _build(tc, nc, x_d, wqkv_d, temp_d, wproj_d, bproj_d, out_d):
    import contextlib

    ctx = contextlib.ExitStack()
    with ctx:
        singles = ctx.enter_context(tc.tile_pool(name="singles", bufs=1))
        dram = ctx.enter_context(tc.tile_pool(name="dram", bufs=1, space="DRAM"))
        ps_tr = ctx.enter_context(tc.tile_pool(name="ps_tr", bufs=PSTR_BUFS, space="PSUM"))

        ident_f32 = singles.tile([128, 128], FP32)
        make_identity(nc, ident_f32)
        ident = singles.tile([128, 128], BF16)
        nc.vector.tensor_copy(ident, ident_f32)
        ident96f = ident_f32[0:HD, 0:HD]

        b_all = singles.tile([128, C], FP32)
        nc.gpsimd.dma_start(
            b_all, bass.AP(tensor=bproj_d.tensor, offset=bproj_d.offset,
                           ap=[[0, 128], [1, C]]))
        temp_all = singles.tile([HD, H], FP32)
        nc.gpsimd.dma_start(
            temp_all, bass.AP(tensor=temp_d.tensor, offset=temp_d.offset,
                              ap=[[0, HD], [1, H]]))

        warm = singles.tile([1, 1], FP32)
        nc.vector.memset(warm, 0.5)
        nc.scalar.activation(warm, warm, mybir.ActivationFunctionType.Exp)
        nc.scalar.sqrt(warm, warm)

        cg_accum = singles.tile([HD, H, 288], FP32)
        nc.vector.memset(cg_accum, 0.0)
        attnT = singles.tile([HD, H, HD], BF16)
        vt_all = singles.tile([HD, H, N_TOK], BF16)

        # ---------------- phase 0 + 1 ----------------
        # w_qk plain layout: w_qk_f8[p][:, kk, j] = 64 * w_qkv[512p+j, 128kk+:]
        # (cols 0..767 across the 3 tiles = q rows, 768..1535 = k rows)
        with tc.tile_pool(name="wqk_pool", bufs=1) as wqk_pool, \
             tc.tile_pool(name="xin", bufs=XIN_BUFS) as xin, \
             tc.tile_pool(name="xtp", bufs=XTP_BUFS) as xtp, \
             tc.tile_pool(name="qkp", bufs=QKP_BUFS) as qkp, \
             tc.tile_pool(name="vtsb", bufs=2) as vtsb, \
             tc.tile_pool(name="ps_mm", bufs=PSMM_BUFS, space="PSUM") as ps_mm:
            w_qk_f8 = [wqk_pool.tile([128, KK, 512], FP8, name=f"wqk{p}")
                       for p in range(3)]
            w_vT = wqk_pool.tile([128, KK, C], BF16)   # holds 64*w_v
            vstrips = _vt_strips()
            state = {"vt_sb": None, "qk_pair": []}

            def xstage(t):
                """x load + bf16 pre-cast + PE transpose + bf16/fp8 casts."""
                t0 = t * TOK_TILE
                x_t = xin.tile([128, CHUNKS, C], BF16, name="x_t")
                nc.gpsimd.dma_start(
                    x_t, x_d[t0:t0 + TOK_TILE, :].rearrange(
                        "(c p) f -> p c f", p=128))
                xT_bf = xtp.tile([128, KK, TOK_TILE], BF16, name="xT_bf")
                xT_f8 = xtp.tile([128, KK, TOK_TILE], FP8, name="xT_f8")
                for kk in range(KK):
                    xps = ps_tr.tile([128, TOK_TILE], BF16, name="xps",
                                     tag="tr")
                    for c in range(CHUNKS):
                        nc.tensor.transpose(
                            xps[:, c * 128:(c + 1) * 128],
                            x_t[:, c, kk * 128:(kk + 1) * 128], ident)
                    if kk % 2 == 0:
                        nc.vector.tensor_copy(xT_bf[:, kk, :], xps)
                        nc.scalar.copy(xT_f8[:, kk, :], xps)
                    else:
                        nc.scalar.copy(xT_bf[:, kk, :], xps)
                        nc.vector.tensor_copy(xT_f8[:, kk, :], xps)
                return xT_bf, xT_f8

            def mmstage(t, xT_bf, xT_f8, inline_vt=False):
                """qk + vT matmuls, vt strips, covariance for one tile."""
                t0 = t * TOK_TILE
                qk_t = qkp.tile([128, CHUNKS, 1536], FP8, name="qk_t")

                # qk = xT.T @ w_qk (token-major; fp8 DoubleRow pairs),
                # interleaved with the vT groups of the previous tile's
                # cadence via emission order (qk pairs then one vt handled
                # in vtstage) — here just emit the 12 psum groups
                def qk_group(c, p):
                    mmps = ps_mm.tile([128, 512], FP32, name="mmps",
                                      tag="s")
                    for i in range(KK // 2):
                        nc.tensor.matmul(
                            mmps,
                            xT_f8[:, 2 * i:2 * i + 2,
                                  c * 128:(c + 1) * 128],
                            w_qk_f8[p][:, 2 * i:2 * i + 2, :],
                            start=(i == 0), stop=(i == KK // 2 - 1),
                            perf_mode=DR)
                    if (c * 3 + p) % 2 == 0:
                        nc.scalar.mul(
                            qk_t[:, c, p * 512:(p + 1) * 512], mmps, C_QK)
                    else:
                        nc.vector.tensor_scalar_mul(
                            qk_t[:, c, p * 512:(p + 1) * 512], mmps, C_QK)

                if inline_vt:
                    if t % 2 == 0:
                        state["vt_sb"] = vtsb.tile(
                            [128, KK, 2 * TOK_TILE], BF16, name="vt_sb")
                    vt_sb = state["vt_sb"]
                    half = (t % 2) * TOK_TILE
                    for j in range(KK):
                        qk_group(*divmod(2 * j, 3))
                        qk_group(*divmod(2 * j + 1, 3))
                        vps = ps_mm.tile([128, TOK_TILE], FP32, name="vps",
                                         tag="s")
                        for kk in range(KK):
                            nc.tensor.matmul(
                                vps, w_vT[:, kk, j * 128:(j + 1) * 128],
                                xT_bf[:, kk, :],
                                start=(kk == 0), stop=(kk == KK - 1))
                        if j % 2 == 0:
                            nc.scalar.mul(
                                vt_sb[:, j, half:half + TOK_TILE], vps,
                                1.0 / S_W)
                        else:
                            nc.vector.tensor_scalar_mul(
                                vt_sb[:, j, half:half + TOK_TILE], vps,
                                1.0 / S_W)
                    if t % 2 == 1:
                        tp0 = (t - 1) * TOK_TILE
                        for si, (m, p0, run, h, d0) in enumerate(vstrips):
                            src = vt_sb[p0:p0 + run, m, :]
                            dst = vt_all[d0:d0 + run, h,
                                         tp0:tp0 + 2 * TOK_TILE]
                            if si % 3 == 0:
                                nc.sync.dma_start(dst, src)
                            elif si % 3 == 1:
                                nc.scalar.dma_start(dst, src)
                            else:
                                nc.gpsimd.dma_start(dst, src)
                else:
                    for c in range(CHUNKS):
                        for p in range(3):
                            qk_group(c, p)

                # covariance + Gram over a pair of tiles: per head
                # [Gq | Gk | C] = [q'q | k'k | q'k], DoubleRow chunk pairs.
                # Emitted before vT on odd tiles so the final flush (and the
                # phase-2 chain it gates) overlaps the last tile's vT matmuls.
                state["qk_pair"].append(qk_t)
                if t % 2 == 1:
                    qk_pair = state["qk_pair"]
                    for h in range(H):
                        cg_ps = ps_mm.tile([HD, 288], FP32, name="cg_ps",
                                           tag="s")
                        np_ = 2 * len(qk_pair)
                        for i in range(np_):
                            qkx = qk_pair[i // 2]
                            lo = (i % 2) * 2
                            q_sl = qkx[:, lo:lo + 2, HD * h:HD * h + HD]
                            k_sl = qkx[:, lo:lo + 2,
                                       C + HD * h:C + HD * h + HD]
                            nc.tensor.matmul(
                                cg_ps[:, 0:HD], q_sl, q_sl,
                                start=(i == 0), stop=False, perf_mode=DR)
                            nc.tensor.matmul(
                                cg_ps[:, HD:2 * HD], k_sl, k_sl,
                                start=False, stop=False, perf_mode=DR)
                            nc.tensor.matmul(
                                cg_ps[:, 2 * HD:3 * HD], q_sl, k_sl,
                                start=False, stop=(i == np_ - 1),
                                perf_mode=DR)
                        nc.vector.tensor_add(
                            cg_accum[:, h, :], cg_ps, cg_accum[:, h, :])
                    state["qk_pair"] = []

            def vtstage(t, xT_bf, act_only=False):
                # vT = w_vT.T @ xT_bf (feature-major, c-major blocks);
                # batched over pairs of tiles to halve strip-DMA count
                if t % 2 == 0:
                    state["vt_sb"] = vtsb.tile([128, KK, 2 * TOK_TILE], BF16,
                                               name="vt_sb")
                vt_sb = state["vt_sb"]
                half = (t % 2) * TOK_TILE
                for m in range(KK):
                    vps = ps_mm.tile([128, TOK_TILE], FP32, name="vps",
                                     tag="s")
                    for kk in range(KK):
                        nc.tensor.matmul(
                            vps, w_vT[:, kk, m * 128:(m + 1) * 128],
                            xT_bf[:, kk, :],
                            start=(kk == 0), stop=(kk == KK - 1))
                    if act_only or m % 2 == 0:
                        nc.scalar.mul(
                            vt_sb[:, m, half:half + TOK_TILE], vps, 1.0 / S_W)
                    else:
                        nc.vector.tensor_scalar_mul(
                            vt_sb[:, m, half:half + TOK_TILE], vps, 1.0 / S_W)
                if t % 2 == 1:
                    tp0 = (t - 1) * TOK_TILE
                    for si, (m, p0, run, h, d0) in enumerate(vstrips):
                        src = vt_sb[p0:p0 + run, m, :]
                        dst = vt_all[d0:d0 + run, h, tp0:tp0 + 2 * TOK_TILE]
                        if si % 3 == 0:
                            nc.sync.dma_start(dst, src)
                        elif si % 3 == 1:
                            nc.scalar.dma_start(dst, src)
                        else:
                            nc.gpsimd.dma_start(dst, src)

            # w prep in groups of 4 row-blocks: one wide psum + one cast per
            # (group, kk) instead of 4 narrow ones, using the ps_mm banks
            # that sit idle until the first qk matmul. Interleaved with the
            # first two x stages: the first qk matmul only needs w_qk
            # p-block 0 (m 0..3) + xT(0).
            xT01 = [None, None]
            with tc.tile_pool(name="wload", bufs=2) as wload:
                def wprep(grp):
                    w_blk = wload.tile([128, 2, C], BF16, name="w_blk")
                    nc.gpsimd.dma_start(
                        w_blk,
                        wqkv_d[grp * 256:(grp + 1) * 256, :].rearrange(
                            "(b p) f -> p b f", p=128))
                    for kk in range(KK):
                        tps = ps_mm.tile([128, 256], BF16, name="wps",
                                         tag="s")
                        for b in range(2):
                            nc.tensor.transpose(
                                tps[:, b * 128:(b + 1) * 128],
                                w_blk[:, b, kk * 128:(kk + 1) * 128], ident)
                        if grp < 6:
                            dst = w_qk_f8[grp // 2][
                                :, kk, (grp % 2) * 256:(grp % 2) * 256 + 256]
                            if (grp + kk) % 2 == 0:
                                nc.vector.tensor_scalar_mul(dst, tps, S_W)
                            else:
                                nc.scalar.mul(dst, tps, S_W)
                        else:
                            base = (grp - 6) * 256
                            dst = w_vT[:, kk, base:base + 256]
                            if kk % 2 == 0:
                                nc.vector.tensor_scalar_mul(dst, tps, S_W)
                            else:
                                nc.scalar.mul(dst, tps, S_W)

                xT01[0] = xstage(0)
                wprep(0)
                wprep(1)
                xT01[1] = xstage(1)
                wprep(2)
                wprep(3)
                for grp in range(4, 9):
                    wprep(grp)

            for t in range(N_TILES):
                xts = xT01[t] if t < 2 else state.pop(("x", t))
                mmstage(t, *xts, inline_vt=(t < 6))
                if t >= 6:
                    state[("xts", t)] = xts
                if t + 2 < N_TILES:
                    state[("x", t + 2)] = xstage(t + 2)

            phase2(nc, tc, singles, dram, ps_tr, cg_accum, attnT, temp_all,
                   ident96f)

            # deferred vT for the last tile pair: fills the PE while the
            # phase-2 DVE/ACT softmax chain runs
            vtstage(6, state.pop(("xts", 6))[0])
            vtstage(7, state.pop(("xts", 7))[0])

        # ---------------- phase 3 pools; w_projT prep emitted first so the
        # PE has work while the DVE/ACT-heavy phase 2 chain runs ----------
        with tc.tile_pool(name="wpp", bufs=1) as wpp, \
             tc.tile_pool(name="wpload", bufs=2) as wpload, \
             tc.tile_pool(name="ot96p", bufs=2) as ot96p, \
             tc.tile_pool(name="otp", bufs=2) as otp, \
             tc.tile_pool(name="yp", bufs=2) as yp, \
             tc.tile_pool(name="ps_o", bufs=3, space="PSUM") as ps_o, \
             tc.tile_pool(name="ps_y", bufs=3, space="PSUM") as ps_y:
            # w_proj (cout, c) -> w_projT128 [128, m, cout] (dense c-major)
            w_projT = wpp.tile([128, KK, C], BF16)
            for n in range(KK):
                wp_blk = wpload.tile([128, C], BF16, name="wp_blk")
                nc.gpsimd.dma_start(wp_blk, wproj_d[n * 128:(n + 1) * 128, :])
                for m in range(KK):
                    tps2 = ps_tr.tile([128, 128], BF16, name="tps2", tag="tr")
                    nc.tensor.transpose(
                        tps2, wp_blk[:, m * 128:(m + 1) * 128], ident)
                    if (n + m) % 2 == 0:
                        nc.vector.tensor_copy(
                            w_projT[:, m, n * 128:(n + 1) * 128], tps2)
                    else:
                        nc.scalar.copy(
                            w_projT[:, m, n * 128:(n + 1) * 128], tps2)

            # ---------------- phase 3: attn@v + proj, sw-pipelined over
            # 2-tile groups (halves the relayout strip-DMA count) ----------
            ostrips = _ot_strips()
            T2 = 2 * TOK_TILE

            def attnv_group(g):
                g0 = g * T2
                ot96 = ot96p.tile([HD, H, T2], BF16, name="ot96")
                otsb = otp.tile([128, KK, T2], BF16, name="otsb")
                for half in range(2):
                    t0 = g0 + half * TOK_TILE
                    for h in range(H):
                        ops_ = ps_o.tile([HD, TOK_TILE], FP32, name="ops_")
                        nc.tensor.matmul(ops_, attnT[:, h, :],
                                         vt_all[:, h, t0:t0 + TOK_TILE],
                                         start=True, stop=True)
                        hh0 = half * TOK_TILE
                        if h % 2 == 0:
                            nc.vector.tensor_copy(
                                ot96[:, h, hh0:hh0 + TOK_TILE], ops_)
                        else:
                            nc.scalar.copy(
                                ot96[:, h, hh0:hh0 + TOK_TILE], ops_)
                for si, (h, d0, run, m, p0) in enumerate(ostrips):
                    src = ot96[d0:d0 + run, h, :]
                    dst = otsb[p0:p0 + run, m, :]
                    if si % 3 == 0:
                        nc.sync.dma_start(dst, src)
                    elif si % 3 == 1:
                        nc.scalar.dma_start(dst, src)
                    else:
                        nc.gpsimd.dma_start(dst, src)
                return otsb

            def proj_group(g, otsb, fine=False):
                np_ = 2 * CHUNKS if fine else CHUNKS
                nc_pp = (2 * CHUNKS) // np_
                for piece in range(np_):
                    t0 = g * T2 + piece * nc_pp * 128
                    y_t = yp.tile([128, nc_pp, C], FP32, name="y_t")
                    for c in range(nc_pp):
                        cc = piece * nc_pp + c
                        for (off, width) in ((0, 512), (512, 256)):
                            yps = ps_y.tile([128, 512], FP32, name="yps")
                            for m in range(KK):
                                nc.tensor.matmul(
                                    yps[:, :width],
                                    otsb[:, m, cc * 128:(cc + 1) * 128],
                                    w_projT[:, m, off:off + width],
                                    start=(m == 0), stop=(m == KK - 1))
                            nc.vector.tensor_tensor(
                                y_t[:, c, off:off + width], yps[:, :width],
                                b_all[:, off:off + width], mybir.AluOpType.add)
                    nc.scalar.dma_start(
                        out_d[t0:t0 + nc_pp * 128, :].rearrange(
                            "(c p) f -> p c f", p=128),
                        y_t)

            pend = None
            for g in range(N_TILES // 2):
                cur = attnv_group(g)
                if pend is not None:
                    proj_group(*pend)
                pend = (g, cur)
            proj_group(*pend)


def _get_nc():
    global _CACHED_NC
    if _CACHED_NC is None:
        _CACHED_NC = build_nc()
    return _CACHED_NC


def kernel(x, w_qkv, temperature, w_proj, b_proj):
    nc = _get_nc()
    x = np.ascontiguousarray(np.asarray(x, dtype=np.float32))
    in_maps = []
    for b in range(8):
        in_maps.append({
            "x": x[b],
            "w_qkv": np.asarray(w_qkv, dtype=np.float32),
            "temperature": np.asarray(temperature, dtype=np.float32),
            "w_proj": np.asarray(w_proj, dtype=np.float32),
            "b_proj": np.asarray(b_proj, dtype=np.float32),
        })
    res = run_bass_kernel_spmd(nc, in_maps, core_ids=list(range(8)))
    return np.stack([r["out"] for r in res.results], axis=0)


# revision 59
# speedup vs baseline: 1.0290x; 1.0200x over previous
"""CrossCovarianceAttn Trainium2 kernel.

Data-parallel over B=8 across 8 NeuronCores; each core runs the full model on
one batch element.

Numerics: the q/k projection and the per-head covariance/Gram matmuls run in
fp8e4m3 with perf_mode=DoubleRow (two 128-row k-tiles per instruction, 0.5
cyc/row) — safe because q,k are l2-normalized over the token dim downstream,
which cancels the fp8 scaling exactly and the softmax logits are small
(|logit| <= temperature by Cauchy-Schwarz, which also lets softmax skip the
max-subtraction). w_q/w_k/w_v are scaled x64 so fp8/bf16 see them in their
normal range; v carries the x64 until the psum->sbuf cast divides it back
out. The v projection, attn@v and the output projection run in bf16 with the
projection contracting dense 128-row c-blocks.

x and the weights are loaded with GpSimd (SWDGE) cast-DMAs that convert
fp32->bf16 in flight (verified bit-exact on device), halving their DMA time;
all PE transposes then run on bf16 data with a bf16 identity (1.0 cyc/row).

Cross-partition relayouts use sbuf->sbuf strip DMAs (compute engines cannot
cross partitions): vT (c-major [128,6,N] from the PE) -> vt_all [96,H,N]
per-head layout for attn@v, and attn@v output (per-head [96,H,N]) ->
otsb [128,6,N] dense c-major for the projection. Both are batched over
2-tile pairs to halve the per-DMA HWDGE fixed cost.

Pipeline per core:
  phase 0: w_qkv -> w_qk_f8 (plain layout, 3 per-512-col tiles so the first
           qk matmul only waits on a third of the prep) + w_vT bf16 (x64),
           interleaved with the first two x stages
  phase 1: per 512-token tile: PE-transpose x -> xT psum; cast to xT_bf +
           xT_f8 (DVE/ACT); qk = xT_f8.T @ w_qk_f8 (DoubleRow) -> qk_t fp8;
           covariance+Gram (3 DoubleRow matmuls per head: Gq | Gk | C)
           accumulated over pairs of tiles into cg_accum fp32; vT = w_vT.T @
           xT_bf -> strip-DMA into vt_all (deferred for the last tile pair
           so it fills the PE during the phase-2 softmax chain)
  phase 2: fused Gram-diagonal extract -> 1/max(||.||,eps); batched all-head
           softmax (no max-sub); transpose attn -> attnT bf16
  phase 3 (sw-pipelined over 2-tile groups): attn_h @ vt_all_h -> ot96 ->
           strip-DMA -> otsb; y = otsb.T @ w_projT + b -> out
"""
import os
import sys

sys.path.insert(0, "/opt/trn_rl_repo")

import numpy as np

import concourse.bass as bass
import concourse.mybir as mybir
import concourse.tile as tile
from concourse import bacc
from concourse.bass_utils import run_bass_kernel_spmd
from concourse.masks import make_identity

FP32 = mybir.dt.float32
FP32R = mybir.dt.float32r
BF16 = mybir.dt.bfloat16
FP8 = mybir.dt.float8e4
DR = mybir.MatmulPerfMode.DoubleRow

N_TOK = 4096
C = 768
H = 8
HD = 96
C3 = 3 * C
TOK_TILE = 512
N_TILES = N_TOK // TOK_TILE
CHUNKS = TOK_TILE // 128
KK = C // 128
EPS = 1e-12

S_W = 64.0           # w_q/w_k (and w_v) -> fp8/bf16 scale
C_QK = 26.0 / 35.5   # qk psum (scaled x64) -> fp8 scale

_CACHED_NC = None
XTP_BUFS = int(os.environ.get("XTP_BUFS", "3"))
QKP_BUFS = int(os.environ.get("QKP_BUFS", "3"))
PSTR_BUFS = int(os.environ.get("PSTR_BUFS", "2"))
PSMM_BUFS = int(os.environ.get("PSMM_BUFS", "6"))
XIN_BUFS = int(os.environ.get("XIN_BUFS", "3"))


def _vt_strips():
    """Strips (m, p0, run, h, d0): vt_sb partition p of block m holds
    v-feature c = 128m + p = 96h + d -> vt_all partition d, head h."""
    strips = []
    for m in range(KK):
        c0 = 128 * m
        p = 0
        while p < 128:
            h, d = divmod(c0 + p, HD)
            run = min(128 - p, HD - d)
            strips.append((m, p, run, h, d))
            p += run
    return strips


def _ot_strips():
    """Strips (h, d0, run, m, p0): attnv psum for head h, row d holds
    out-feature c = 96h + d -> otsb128 partition p = c % 128, block m."""
    strips = []
    for h in range(H):
        c0 = HD * h
        d = 0
        while d < HD:
            m, p = divmod(c0 + d, 128)
            run = min(HD - d, 128 - p)
            strips.append((h, d, run, m, p))
            d += run
    return strips


def phase2(nc, tc, singles, dram, ps_tr, cg_accum, attnT, temp_all, ident96f):
    """Norms + batched all-head softmax -> attnT (bf16).

    cg_accum[:, h, :]: [0:96] Gq, [96:192] Gk, [192:288] C. The Gq|Gk
    adjacency lets one mask-mult + one reduce grab both sets of diagonals.
    |logit| <= temperature (Cauchy-Schwarz on unit vectors), so exp runs
    directly with no max-subtraction.
    """
    import concourse.mybir as mybir

    identb = ident96f[:, None, None, :].to_broadcast((HD, H, 2, HD))
    sq = singles.tile([HD, H, 2], FP32)
    scr = singles.tile([HD, H, 2, HD], FP32)
    nc.vector.tensor_tensor(
        scr, cg_accum[:, :, 0:2 * HD].rearrange(
            "d h (two e) -> d h two e", two=2),
        identb, mybir.AluOpType.mult)
    nc.vector.reduce_sum(sq[:, :, :, None], scr, axis=mybir.AxisListType.X)

    nrm = singles.tile([HD, H, 2], FP32)
    nc.scalar.sqrt(nrm, sq)
    nc.vector.tensor_scalar_max(nrm, nrm, EPS)
    rnorm = singles.tile([HD, H, 2], FP32)
    nc.vector.reciprocal(rnorm, nrm)
    rq = singles.tile([HD, H], FP32)
    nc.vector.tensor_tensor(rq, rnorm[:, :, 0], temp_all,
                            mybir.AluOpType.mult)

    # rk to the free dim: store h-major to DRAM, broadcast-read back
    rk_scr = dram.tile([H, HD], FP32)
    nc.sync.dma_start(
        bass.AP(tensor=rk_scr.tensor, offset=rk_scr.offset,
                ap=[[1, HD], [HD, H]]),
        rnorm[:, :, 1])
    rk_all = singles.tile([HD, H, HD], FP32)
    nc.sync.dma_start(
        rk_all, bass.AP(tensor=rk_scr.tensor, offset=rk_scr.offset,
                        ap=[[0, HD], [1, H * HD]]))

    attL = singles.tile([HD, H, HD], FP32)
    nc.vector.tensor_tensor(
        attL, cg_accum[:, :, 2 * HD:3 * HD],
        rq[:, :, None].to_broadcast((HD, H, HD)), mybir.AluOpType.mult)
    nc.vector.tensor_tensor(attL, attL, rk_all, mybir.AluOpType.mult)
    nc.scalar.activation(attL, attL, mybir.ActivationFunctionType.Exp)
    sea = singles.tile([HD, H, 1], FP32)
    nc.vector.reduce_sum(sea, attL, axis=mybir.AxisListType.X)
    rsea = singles.tile([HD, H, 1], FP32)
    nc.vector.reciprocal(rsea, sea)
    nc.vector.tensor_tensor(
        attnT, attL, rsea.to_broadcast((HD, H, HD)), mybir.AluOpType.mult)


def build_nc():
    nc = bacc.Bacc("TRN2", target_bir_lowering=False, debug=False, num_devices=8)

    x_d = nc.dram_tensor("x", (N_TOK, C), FP32R, kind="ExternalInput").ap()
    wqkv_d = nc.dram_tensor("w_qkv", (C3, C), FP32R, kind="ExternalInput").ap()
    temp_d = nc.dram_tensor("temperature", (H, 1, 1), FP32, kind="ExternalInput").ap()
    wproj_d = nc.dram_tensor("w_proj", (C, C), FP32R, kind="ExternalInput").ap()
    bproj_d = nc.dram_tensor("b_proj", (C,), FP32, kind="ExternalInput").ap()
    out_d = nc.dram_tensor("out", (N_TOK, C), FP32, kind="ExternalOutput").ap()

    with tile.TileContext(nc) as tc:
        _build(tc, nc, x_d, wqkv_d, temp_d, wproj_d, bproj_d, out_d)
    nc.compile()
    return nc


def _build(tc, nc, x_d, wqkv_d, temp_d, wproj_d, bproj_d, out_d):
    import contextlib

    ctx = contextlib.ExitStack()
    with ctx:
        singles = ctx.enter_context(tc.tile_pool(name="singles", bufs=1))
        dram = ctx.enter_context(tc.tile_pool(name="dram", bufs=1, space="DRAM"))
        ps_tr = ctx.enter_context(tc.tile_pool(name="ps_tr", bufs=PSTR_BUFS, space="PSUM"))

        ident_f32 = singles.tile([128, 128], FP32)
        make_identity(nc, ident_f32)
        ident = singles.tile([128, 128], BF16)
        nc.vector.tensor_copy(ident, ident_f32)
        ident96f = ident_f32[0:HD, 0:HD]

        b_all = singles.tile([128, C], FP32)
        nc.gpsimd.dma_start(
            b_all, bass.AP(tensor=bproj_d.tensor, offset=bproj_d.offset,
                           ap=[[0, 128], [1, C]]))
        temp_all = singles.tile([HD, H], FP32)
        nc.gpsimd.dma_start(
            temp_all, bass.AP(tensor=temp_d.tensor, offset=temp_d.offset,
                              ap=[[0, HD], [1, H]]))

        warm = singles.tile([1, 1], FP32)
        nc.vector.memset(warm, 0.5)
        nc.scalar.activation(warm, warm, mybir.ActivationFunctionType.Exp)
        nc.scalar.sqrt(warm, warm)

        cg_accum = singles.tile([HD, H, 288], FP32)
        nc.vector.memset(cg_accum, 0.0)
        attn_bf = singles.tile([HD, H, HD], BF16)
        vt128 = singles.tile([128, KK, N_TOK], BF16)

        # ---------------- phase 0 + 1 ----------------
        # w_qk plain layout: w_qk_f8[p][:, kk, j] = 64 * w_qkv[512p+j, 128kk+:]
        # (cols 0..767 across the 3 tiles = q rows, 768..1535 = k rows)
        with tc.tile_pool(name="wqk_pool", bufs=1) as wqk_pool, \
             tc.tile_pool(name="xin", bufs=XIN_BUFS) as xin, \
             tc.tile_pool(name="xtp", bufs=XTP_BUFS) as xtp, \
             tc.tile_pool(name="qkp", bufs=QKP_BUFS) as qkp, \
             tc.tile_pool(name="ps_mm", bufs=PSMM_BUFS, space="PSUM") as ps_mm:
            w_qk_f8 = [wqk_pool.tile([128, KK, 512], FP8, name=f"wqk{p}")
                       for p in range(3)]
            w_vT = wqk_pool.tile([128, KK, C], BF16)   # holds 64*w_v
            vstrips = _vt_strips()
            state = {"vt_sb": None, "qk_pair": []}

            def xstage(t):
                """x load + bf16 pre-cast + PE transpose + bf16/fp8 casts."""
                t0 = t * TOK_TILE
                x_t = xin.tile([128, CHUNKS, C], BF16, name="x_t")
                nc.gpsimd.dma_start(
                    x_t, x_d[t0:t0 + TOK_TILE, :].rearrange(
                        "(c p) f -> p c f", p=128))
                xT_bf = xtp.tile([128, KK, TOK_TILE], BF16, name="xT_bf")
                xT_f8 = xtp.tile([128, KK, TOK_TILE], FP8, name="xT_f8")
                for kk in range(KK):
                    xps = ps_tr.tile([128, TOK_TILE], BF16, name="xps",
                                     tag="tr")
                    for c in range(CHUNKS):
                        nc.tensor.transpose(
                            xps[:, c * 128:(c + 1) * 128],
                            x_t[:, c, kk * 128:(kk + 1) * 128], ident)
                    if kk % 2 == 0:
                        nc.vector.tensor_copy(xT_bf[:, kk, :], xps)
                        nc.scalar.copy(xT_f8[:, kk, :], xps)
                    else:
                        nc.scalar.copy(xT_bf[:, kk, :], xps)
                        nc.vector.tensor_copy(xT_f8[:, kk, :], xps)
                return xT_bf, xT_f8

            def mmstage(t, xT_bf, xT_f8, inline_vt=False):
                """qk + vT matmuls, vt strips, covariance for one tile."""
                t0 = t * TOK_TILE
                qk_t = qkp.tile([128, CHUNKS, 1536], FP8, name="qk_t")

                # qk = xT.T @ w_qk (token-major; fp8 DoubleRow pairs),
                # interleaved with the vT groups of the previous tile's
                # cadence via emission order (qk pairs then one vt handled
                # in vtstage) — here just emit the 12 psum groups
                def qk_group(c, p):
                    mmps = ps_mm.tile([128, 512], FP32, name="mmps",
                                      tag="s")
                    for i in range(KK // 2):
                        nc.tensor.matmul(
                            mmps,
                            xT_f8[:, 2 * i:2 * i + 2,
                                  c * 128:(c + 1) * 128],
                            w_qk_f8[p][:, 2 * i:2 * i + 2, :],
                            start=(i == 0), stop=(i == KK // 2 - 1),
                            perf_mode=DR)
                    if (c * 3 + p) % 2 == 0:
                        nc.scalar.mul(
                            qk_t[:, c, p * 512:(p + 1) * 512], mmps, C_QK)
                    else:
                        nc.vector.tensor_scalar_mul(
                            qk_t[:, c, p * 512:(p + 1) * 512], mmps, C_QK)

                if inline_vt:
                    t0v = t * TOK_TILE
                    for j in range(KK):
                        qk_group(*divmod(2 * j, 3))
                        qk_group(*divmod(2 * j + 1, 3))
                        vps = ps_mm.tile([128, TOK_TILE], FP32, name="vps",
                                         tag="s")
                        for kk in range(KK):
                            nc.tensor.matmul(
                                vps, w_vT[:, kk, j * 128:(j + 1) * 128],
                                xT_bf[:, kk, :],
                                start=(kk == 0), stop=(kk == KK - 1))
                        if j % 2 == 0:
                            nc.scalar.mul(
                                vt128[:, j, t0v:t0v + TOK_TILE], vps,
                                1.0 / S_W)
                        else:
                            nc.vector.tensor_scalar_mul(
                                vt128[:, j, t0v:t0v + TOK_TILE], vps,
                                1.0 / S_W)
                else:
                    for c in range(CHUNKS):
                        for p in range(3):
                            qk_group(c, p)

                # covariance + Gram over a pair of tiles: per head
                # [Gq | Gk | C] = [q'q | k'k | q'k], DoubleRow chunk pairs.
                # Emitted before vT on odd tiles so the final flush (and the
                # phase-2 chain it gates) overlaps the last tile's vT matmuls.
                state["qk_pair"].append(qk_t)
                if t % 2 == 1:
                    qk_pair = state["qk_pair"]
                    for h in range(H):
                        cg_ps = ps_mm.tile([HD, 288], FP32, name="cg_ps",
                                           tag="s")
                        np_ = 2 * len(qk_pair)
                        for i in range(np_):
                            qkx = qk_pair[i // 2]
                            lo = (i % 2) * 2
                            q_sl = qkx[:, lo:lo + 2, HD * h:HD * h + HD]
                            k_sl = qkx[:, lo:lo + 2,
                                       C + HD * h:C + HD * h + HD]
                            nc.tensor.matmul(
                                cg_ps[:, 0:HD], q_sl, q_sl,
                                start=(i == 0), stop=False, perf_mode=DR)
                            nc.tensor.matmul(
                                cg_ps[:, HD:2 * HD], k_sl, k_sl,
                                start=False, stop=False, perf_mode=DR)
                            nc.tensor.matmul(
                                cg_ps[:, 2 * HD:3 * HD], q_sl, k_sl,
                                start=False, stop=(i == np_ - 1),
                                perf_mode=DR)
                        nc.vector.tensor_add(
                            cg_accum[:, h, :], cg_ps, cg_accum[:, h, :])
                    state["qk_pair"] = []

            def vtstage(t, xT_bf):
                # vT = w_vT.T @ xT_bf (feature-major, straight into the
                # c-major vt128 — the fused projection consumes it as-is)
                t0v = t * TOK_TILE
                for m in range(KK):
                    vps = ps_mm.tile([128, TOK_TILE], FP32, name="vps",
                                     tag="s")
                    for kk in range(KK):
                        nc.tensor.matmul(
                            vps, w_vT[:, kk, m * 128:(m + 1) * 128],
                            xT_bf[:, kk, :],
                            start=(kk == 0), stop=(kk == KK - 1))
                    if m % 2 == 0:
                        nc.scalar.mul(
                            vt128[:, m, t0v:t0v + TOK_TILE], vps, 1.0 / S_W)
                    else:
                        nc.vector.tensor_scalar_mul(
                            vt128[:, m, t0v:t0v + TOK_TILE], vps, 1.0 / S_W)

            # w prep in groups of 4 row-blocks: one wide psum + one cast per
            # (group, kk) instead of 4 narrow ones, using the ps_mm banks
            # that sit idle until the first qk matmul. Interleaved with the
            # first two x stages: the first qk matmul only needs w_qk
            # p-block 0 (m 0..3) + xT(0).
            xT01 = [None, None]
            with tc.tile_pool(name="wload", bufs=2) as wload:
                def wprep(grp):
                    w_blk = wload.tile([128, 2, C], BF16, name="w_blk")
                    nc.gpsimd.dma_start(
                        w_blk,
                        wqkv_d[grp * 256:(grp + 1) * 256, :].rearrange(
                            "(b p) f -> p b f", p=128))
                    for kk in range(KK):
                        tps = ps_mm.tile([128, 256], BF16, name="wps",
                                         tag="s")
                        for b in range(2):
                            nc.tensor.transpose(
                                tps[:, b * 128:(b + 1) * 128],
                                w_blk[:, b, kk * 128:(kk + 1) * 128], ident)
                        if grp < 6:
                            dst = w_qk_f8[grp // 2][
                                :, kk, (grp % 2) * 256:(grp % 2) * 256 + 256]
                            if (grp + kk) % 2 == 0:
                                nc.vector.tensor_scalar_mul(dst, tps, S_W)
                            else:
                                nc.scalar.mul(dst, tps, S_W)
                        else:
                            base = (grp - 6) * 256
                            dst = w_vT[:, kk, base:base + 256]
                            if kk % 2 == 0:
                                nc.vector.tensor_scalar_mul(dst, tps, S_W)
                            else:
                                nc.scalar.mul(dst, tps, S_W)

                xT01[0] = xstage(0)
                wprep(0)
                wprep(1)
                xT01[1] = xstage(1)
                wprep(2)
                wprep(3)
                for grp in range(4, 9):
                    wprep(grp)

            for t in range(N_TILES):
                xts = xT01[t] if t < 2 else state.pop(("x", t))
                mmstage(t, *xts, inline_vt=(t < 6))
                if t >= 6:
                    state[("xts", t)] = xts
                if t + 2 < N_TILES:
                    state[("x", t + 2)] = xstage(t + 2)

            phase2(nc, tc, singles, dram, ps_tr, cg_accum, attn_bf, temp_all,
                   ident96f)

            # deferred vT for the last tile pair: fills the PE while the
            # phase-2 DVE/ACT softmax chain runs
            vtstage(6, state.pop(("xts", 6))[0])
            vtstage(7, state.pop(("xts", 7))[0])

        # ---------------- phase 3 pools; w_projT96 prep + the B = attn.T
        # @ w_proj.T fold emitted first: they fill the PE while the DVE/ACT
        # softmax chain runs, and delete attn@v + both strip relayouts ----
        with tc.tile_pool(name="wpp", bufs=1) as wpp, \
             tc.tile_pool(name="wpload", bufs=2) as wpload, \
             tc.tile_pool(name="yp", bufs=2) as yp, \
             tc.tile_pool(name="ps_b", bufs=2, space="PSUM") as ps_b, \
             tc.tile_pool(name="ps_y", bufs=4, space="PSUM") as ps_y:
            # w_proj (cout, c) -> w_projT96 [96, h, cout] (per-head c rows)
            w_projT = wpp.tile([HD, H, C], BF16)
            for n in range(KK):
                wp_blk = wpload.tile([128, C], BF16, name="wp_blk")
                nc.gpsimd.dma_start(wp_blk, wproj_d[n * 128:(n + 1) * 128, :])
                for h in range(H):
                    tps2 = ps_tr.tile([HD, 128], BF16, name="tps2", tag="tr")
                    nc.tensor.transpose(
                        tps2, wp_blk[:, h * HD:(h + 1) * HD], ident)
                    if (n + h) % 2 == 0:
                        nc.vector.tensor_copy(
                            w_projT[:, h, n * 128:(n + 1) * 128], tps2)
                    else:
                        nc.scalar.copy(
                            w_projT[:, h, n * 128:(n + 1) * 128], tps2)

            # B[(h,e), co] = sum_d attn_h[d,e] * wp[co, 96h+d]:
            # per head one [96, 768] result in two psum pieces
            b96 = wpp.tile([HD, H, C], BF16)
            b128 = wpp.tile([128, KK, C], BF16)
            for (off, width) in ((0, 512), (512, 256)):
                for h in range(H):
                    bps = ps_b.tile([HD, 512], FP32, name="bps")
                    nc.tensor.matmul(
                        bps[:, :width], attn_bf[:, h, :],
                        w_projT[:, h, off:off + width],
                        start=True, stop=True)
                    if h % 2 == 0:
                        nc.vector.tensor_copy(
                            b96[:, h, off:off + width], bps[:, :width])
                    else:
                        nc.scalar.copy(
                            b96[:, h, off:off + width], bps[:, :width])
                # relayout rows c=96h+e -> dense 128-blocks, per column half
                # so the first projection psums start before B is complete
                for si, (m, p0, run, h, d0) in enumerate(_vt_strips()):
                    src = b96[d0:d0 + run, h, off:off + width]
                    dst = b128[p0:p0 + run, m, off:off + width]
                    if si % 2 == 0:
                        nc.sync.dma_start(dst, src)
                    else:
                        nc.scalar.dma_start(dst, src)

            # ---------------- phase 3: y = vt128.T @ b128 + b ------------
            for t in range(N_TILES):
                for piece in range(2):
                    t0 = t * TOK_TILE + piece * 256
                    y_t = yp.tile([128, 2, C], FP32, name="y_t")
                    for c in range(2):
                        cc = t * CHUNKS + piece * 2 + c
                        for (off, width) in ((0, 512), (512, 256)):
                            yps = ps_y.tile([128, 512], FP32, name="yps")
                            for m in range(KK):
                                nc.tensor.matmul(
                                    yps[:, :width],
                                    vt128[:, m, cc * 128:(cc + 1) * 128],
                                    b128[:, m, off:off + width],
                                    start=(m == 0), stop=(m == KK - 1))
                            nc.vector.tensor_tensor(
                                y_t[:, c, off:off + width], yps[:, :width],
                                b_all[:, off:off + width], mybir.AluOpType.add)
                    nc.scalar.dma_start(
                        out_d[t0:t0 + 256, :].rearrange(
                            "(c p) f -> p c f", p=128),
                        y_t)


def _get_nc():
    global _CACHED_NC
    if _CACHED_NC is None:
        _CACHED_NC = build_nc()
    return _CACHED_NC


def kernel(x, w_qkv, temperature, w_proj, b_proj):
    nc = _get_nc()
    x = np.ascontiguousarray(np.asarray(x, dtype=np.float32))
    in_maps = []
    for b in range(8):
        in_maps.append({
            "x": x[b],
            "w_qkv": np.asarray(w_qkv, dtype=np.float32),
            "temperature": np.asarray(temperature, dtype=np.float32),
            "w_proj": np.asarray(w_proj, dtype=np.float32),
            "b_proj": np.asarray(b_proj, dtype=np.float32),
        })
    res = run_bass_kernel_spmd(nc, in_maps, core_ids=list(range(8)))
    return np.stack([r["out"] for r in res.results], axis=0)


# revision 61
# speedup vs baseline: 1.0361x; 1.0069x over previous
"""CrossCovarianceAttn Trainium2 kernel.

Data-parallel over B=8 across 8 NeuronCores; each core runs the full model on
one batch element.

Numerics: the q/k projection and the per-head covariance/Gram matmuls run in
fp8e4m3 with perf_mode=DoubleRow (two 128-row k-tiles per instruction, 0.5
cyc/row) — safe because q,k are l2-normalized over the token dim downstream,
which cancels the fp8 scaling exactly and the softmax logits are small
(|logit| <= temperature by Cauchy-Schwarz, which also lets softmax skip the
max-subtraction). w_q/w_k/w_v are scaled x64 so fp8/bf16 see them in their
normal range; the vT psum->sbuf cast divides the x64 back out. The v
projection and the fused attention+output projection run in bf16.

attn@v and the output projection are fused by reassociation:
y = (attn@v).T @ Wp.T  ==  vT.T @ B  with  B[(h,e),co] = sum_d attn_h[d,e] *
wp[co, 96h+d]. B (768x768) is built per-batch on the PE from the softmax
output (~6k cycles) and relayed out per-head->dense-128 with 12 small strip
DMAs; vT is then consumed in the c-major [128,6,N] layout the PE naturally
produces — no attn@v matmuls, no attn transposes, no big strip relayouts.

x and the weights are loaded with GpSimd (SWDGE) cast-DMAs that convert
fp32->bf16 in flight (verified bit-exact on device), halving their DMA time;
all PE transposes run on bf16 data with a bf16 identity (1.0 cyc/row).

Pipeline per core:
  phase 0: w_qkv -> w_qk_f8 (plain layout, 3 per-512-col tiles so the first
           qk matmul only waits on a third of the prep) + w_vT bf16 (x64),
           interleaved with the first two x stages
  phase 1: per 512-token tile: PE-transpose x -> xT psum; cast to xT_bf +
           xT_f8 (DVE/ACT); qk groups (fp8 DoubleRow) interleaved 2:1 with
           vT groups (bf16, written straight into c-major vt128);
           covariance+Gram (3 DoubleRow matmuls per head: Gq | Gk | C)
           accumulated over pairs of tiles into cg_accum fp32; vT for the
           last tile pair is deferred to fill the PE during phase 2
  phase 2: fused Gram-diagonal extract -> 1/max(||.||,eps); batched all-head
           softmax (no max-sub) -> attn_bf
  phase 3: B build (per column half, so the projection starts early);
           y = vt128.T @ b128 + b -> out
"""
import os
import sys

sys.path.insert(0, "/opt/trn_rl_repo")

import numpy as np

import concourse.bass as bass
import concourse.mybir as mybir
import concourse.tile as tile
from concourse import bacc
from concourse.bass_utils import run_bass_kernel_spmd
from concourse.masks import make_identity

FP32 = mybir.dt.float32
FP32R = mybir.dt.float32r
BF16 = mybir.dt.bfloat16
FP8 = mybir.dt.float8e4
DR = mybir.MatmulPerfMode.DoubleRow

N_TOK = 4096
C = 768
H = 8
HD = 96
C3 = 3 * C
TOK_TILE = 512
N_TILES = N_TOK // TOK_TILE
CHUNKS = TOK_TILE // 128
KK = C // 128
EPS = 1e-12

S_W = 64.0           # w_q/w_k (and w_v) -> fp8/bf16 scale
C_QK = 26.0 / 35.5   # qk psum (scaled x64) -> fp8 scale

_CACHED_NC = None
XTP_BUFS = int(os.environ.get("XTP_BUFS", "3"))
QKP_BUFS = int(os.environ.get("QKP_BUFS", "3"))
PSTR_BUFS = int(os.environ.get("PSTR_BUFS", "2"))
PSMM_BUFS = int(os.environ.get("PSMM_BUFS", "6"))
XIN_BUFS = int(os.environ.get("XIN_BUFS", "3"))


def _vt_strips():
    """Strips (m, p0, run, h, d0): vt_sb partition p of block m holds
    v-feature c = 128m + p = 96h + d -> vt_all partition d, head h."""
    strips = []
    for m in range(KK):
        c0 = 128 * m
        p = 0
        while p < 128:
            h, d = divmod(c0 + p, HD)
            run = min(128 - p, HD - d)
            strips.append((m, p, run, h, d))
            p += run
    return strips


def _ot_strips():
    """Strips (h, d0, run, m, p0): attnv psum for head h, row d holds
    out-feature c = 96h + d -> otsb128 partition p = c % 128, block m."""
    strips = []
    for h in range(H):
        c0 = HD * h
        d = 0
        while d < HD:
            m, p = divmod(c0 + d, 128)
            run = min(HD - d, 128 - p)
            strips.append((h, d, run, m, p))
            d += run
    return strips


def phase2(nc, tc, singles, dram, ps_tr, cg_accum, attnT, temp_all, ident96f):
    """Norms + batched all-head softmax -> attnT (bf16).

    cg_accum[:, h, :]: [0:96] Gq, [96:192] Gk, [192:288] C. The Gq|Gk
    adjacency lets one mask-mult + one reduce grab both sets of diagonals.
    |logit| <= temperature (Cauchy-Schwarz on unit vectors), so exp runs
    directly with no max-subtraction.
    """
    import concourse.mybir as mybir

    identb = ident96f[:, None, None, :].to_broadcast((HD, H, 2, HD))
    sq = singles.tile([HD, H, 2], FP32)
    scr = singles.tile([HD, H, 2, HD], FP32)
    nc.vector.tensor_tensor(
        scr, cg_accum[:, :, 0:2 * HD].rearrange(
            "d h (two e) -> d h two e", two=2),
        identb, mybir.AluOpType.mult)
    nc.vector.reduce_sum(sq[:, :, :, None], scr, axis=mybir.AxisListType.X)

    nrm = singles.tile([HD, H, 2], FP32)
    nc.scalar.sqrt(nrm, sq)
    nc.vector.tensor_scalar_max(nrm, nrm, EPS)
    rnorm = singles.tile([HD, H, 2], FP32)
    nc.vector.reciprocal(rnorm, nrm)
    rq = singles.tile([HD, H], FP32)
    nc.vector.tensor_tensor(rq, rnorm[:, :, 0], temp_all,
                            mybir.AluOpType.mult)

    # rk to the free dim: store h-major to DRAM, broadcast-read back
    rk_scr = dram.tile([H, HD], FP32)
    nc.sync.dma_start(
        bass.AP(tensor=rk_scr.tensor, offset=rk_scr.offset,
                ap=[[1, HD], [HD, H]]),
        rnorm[:, :, 1])
    rk_all = singles.tile([HD, H, HD], FP32)
    nc.sync.dma_start(
        rk_all, bass.AP(tensor=rk_scr.tensor, offset=rk_scr.offset,
                        ap=[[0, HD], [1, H * HD]]))

    attL = singles.tile([HD, H, HD], FP32)
    nc.vector.tensor_tensor(
        attL, cg_accum[:, :, 2 * HD:3 * HD],
        rq[:, :, None].to_broadcast((HD, H, HD)), mybir.AluOpType.mult)
    nc.vector.tensor_tensor(attL, attL, rk_all, mybir.AluOpType.mult)
    nc.scalar.activation(attL, attL, mybir.ActivationFunctionType.Exp)
    sea = singles.tile([HD, H, 1], FP32)
    nc.vector.reduce_sum(sea, attL, axis=mybir.AxisListType.X)
    rsea = singles.tile([HD, H, 1], FP32)
    nc.vector.reciprocal(rsea, sea)
    nc.vector.tensor_tensor(
        attnT, attL, rsea.to_broadcast((HD, H, HD)), mybir.AluOpType.mult)


def build_nc():
    nc = bacc.Bacc("TRN2", target_bir_lowering=False, debug=False, num_devices=8)

    x_d = nc.dram_tensor("x", (N_TOK, C), FP32R, kind="ExternalInput").ap()
    wqkv_d = nc.dram_tensor("w_qkv", (C3, C), FP32R, kind="ExternalInput").ap()
    temp_d = nc.dram_tensor("temperature", (H, 1, 1), FP32, kind="ExternalInput").ap()
    wproj_d = nc.dram_tensor("w_proj", (C, C), FP32R, kind="ExternalInput").ap()
    bproj_d = nc.dram_tensor("b_proj", (C,), FP32, kind="ExternalInput").ap()
    out_d = nc.dram_tensor("out", (N_TOK, C), FP32, kind="ExternalOutput").ap()

    with tile.TileContext(nc) as tc:
        _build(tc, nc, x_d, wqkv_d, temp_d, wproj_d, bproj_d, out_d)
    nc.compile()
    return nc


def _build(tc, nc, x_d, wqkv_d, temp_d, wproj_d, bproj_d, out_d):
    import contextlib

    ctx = contextlib.ExitStack()
    with ctx:
        singles = ctx.enter_context(tc.tile_pool(name="singles", bufs=1))
        dram = ctx.enter_context(tc.tile_pool(name="dram", bufs=1, space="DRAM"))
        ps_tr = ctx.enter_context(tc.tile_pool(name="ps_tr", bufs=PSTR_BUFS, space="PSUM"))

        ident_f32 = singles.tile([128, 128], FP32)
        make_identity(nc, ident_f32)
        ident = singles.tile([128, 128], BF16)
        nc.vector.tensor_copy(ident, ident_f32)
        ident96f = ident_f32[0:HD, 0:HD]

        b_all = singles.tile([128, C], FP32)
        nc.gpsimd.dma_start(
            b_all, bass.AP(tensor=bproj_d.tensor, offset=bproj_d.offset,
                           ap=[[0, 128], [1, C]]))
        temp_all = singles.tile([HD, H], FP32)
        nc.gpsimd.dma_start(
            temp_all, bass.AP(tensor=temp_d.tensor, offset=temp_d.offset,
                              ap=[[0, HD], [1, H]]))

        warm = singles.tile([1, 1], FP32)
        nc.vector.memset(warm, 0.5)
        nc.scalar.activation(warm, warm, mybir.ActivationFunctionType.Exp)
        nc.scalar.sqrt(warm, warm)

        cg_accum = singles.tile([HD, H, 288], FP32)
        nc.vector.memset(cg_accum, 0.0)
        attn_bf = singles.tile([HD, H, HD], BF16)
        vt128 = singles.tile([128, KK, N_TOK], BF16)

        # ---------------- phase 0 + 1 ----------------
        # w_qk plain layout: w_qk_f8[p][:, kk, j] = 64 * w_qkv[512p+j, 128kk+:]
        # (cols 0..767 across the 3 tiles = q rows, 768..1535 = k rows)
        with tc.tile_pool(name="wqk_pool", bufs=1) as wqk_pool, \
             tc.tile_pool(name="xin", bufs=XIN_BUFS) as xin, \
             tc.tile_pool(name="xtp", bufs=XTP_BUFS) as xtp, \
             tc.tile_pool(name="qkp", bufs=QKP_BUFS) as qkp, \
             tc.tile_pool(name="ps_mm", bufs=PSMM_BUFS, space="PSUM") as ps_mm:
            w_qk_f8 = [wqk_pool.tile([128, KK, 512], FP8, name=f"wqk{p}")
                       for p in range(3)]
            w_vT = wqk_pool.tile([128, KK, C], BF16)   # holds 64*w_v
            vstrips = _vt_strips()
            state = {"vt_sb": None, "qk_pair": []}

            def xstage(t):
                """x load + bf16 pre-cast + PE transpose + bf16/fp8 casts."""
                t0 = t * TOK_TILE
                x_t = xin.tile([128, CHUNKS, C], BF16, name="x_t")
                nc.gpsimd.dma_start(
                    x_t, x_d[t0:t0 + TOK_TILE, :].rearrange(
                        "(c p) f -> p c f", p=128))
                xT_bf = xtp.tile([128, KK, TOK_TILE], BF16, name="xT_bf")
                xT_f8 = xtp.tile([128, KK, TOK_TILE], FP8, name="xT_f8")
                for kk in range(KK):
                    xps = ps_tr.tile([128, TOK_TILE], BF16, name="xps",
                                     tag="tr")
                    for c in range(CHUNKS):
                        nc.tensor.transpose(
                            xps[:, c * 128:(c + 1) * 128],
                            x_t[:, c, kk * 128:(kk + 1) * 128], ident)
                    if kk % 2 == 0:
                        nc.vector.tensor_copy(xT_bf[:, kk, :], xps)
                        nc.scalar.copy(xT_f8[:, kk, :], xps)
                    else:
                        nc.scalar.copy(xT_bf[:, kk, :], xps)
                        nc.vector.tensor_copy(xT_f8[:, kk, :], xps)
                return xT_bf, xT_f8

            def mmstage(t, xT_bf, xT_f8, inline_vt=False):
                """qk + vT matmuls, vt strips, covariance for one tile."""
                t0 = t * TOK_TILE
                qk_t = qkp.tile([128, CHUNKS, 1536], FP8, name="qk_t")

                # qk = xT.T @ w_qk (token-major; fp8 DoubleRow pairs),
                # interleaved with the vT groups of the previous tile's
                # cadence via emission order (qk pairs then one vt handled
                # in vtstage) — here just emit the 12 psum groups
                def qk_group(c, p):
                    mmps = ps_mm.tile([128, 512], FP32, name="mmps",
                                      tag="s")
                    for i in range(KK // 2):
                        nc.tensor.matmul(
                            mmps,
                            xT_f8[:, 2 * i:2 * i + 2,
                                  c * 128:(c + 1) * 128],
                            w_qk_f8[p][:, 2 * i:2 * i + 2, :],
                            start=(i == 0), stop=(i == KK // 2 - 1),
                            perf_mode=DR)
                    if (c * 3 + p) % 2 == 0:
                        nc.scalar.mul(
                            qk_t[:, c, p * 512:(p + 1) * 512], mmps, C_QK)
                    else:
                        nc.vector.tensor_scalar_mul(
                            qk_t[:, c, p * 512:(p + 1) * 512], mmps, C_QK)

                if inline_vt:
                    t0v = t * TOK_TILE
                    for j in range(KK):
                        qk_group(*divmod(2 * j, 3))
                        qk_group(*divmod(2 * j + 1, 3))
                        vps = ps_mm.tile([128, TOK_TILE], FP32, name="vps",
                                         tag="s")
                        for kk in range(KK):
                            nc.tensor.matmul(
                                vps, w_vT[:, kk, j * 128:(j + 1) * 128],
                                xT_bf[:, kk, :],
                                start=(kk == 0), stop=(kk == KK - 1))
                        if j % 2 == 0:
                            nc.scalar.mul(
                                vt128[:, j, t0v:t0v + TOK_TILE], vps,
                                1.0 / S_W)
                        else:
                            nc.vector.tensor_scalar_mul(
                                vt128[:, j, t0v:t0v + TOK_TILE], vps,
                                1.0 / S_W)
                else:
                    for c in range(CHUNKS):
                        for p in range(3):
                            qk_group(c, p)

                # covariance + Gram over a pair of tiles: per head
                # [Gq | Gk | C] = [q'q | k'k | q'k], DoubleRow chunk pairs.
                # Emitted before vT on odd tiles so the final flush (and the
                # phase-2 chain it gates) overlaps the last tile's vT matmuls.
                state["qk_pair"].append(qk_t)
                if t % 2 == 1:
                    qk_pair = state["qk_pair"]
                    for h in range(H):
                        cg_ps = ps_mm.tile([HD, 288], FP32, name="cg_ps",
                                           tag="s")
                        np_ = 2 * len(qk_pair)
                        for i in range(np_):
                            qkx = qk_pair[i // 2]
                            lo = (i % 2) * 2
                            q_sl = qkx[:, lo:lo + 2, HD * h:HD * h + HD]
                            k_sl = qkx[:, lo:lo + 2,
                                       C + HD * h:C + HD * h + HD]
                            nc.tensor.matmul(
                                cg_ps[:, 0:HD], q_sl, q_sl,
                                start=(i == 0), stop=False, perf_mode=DR)
                            nc.tensor.matmul(
                                cg_ps[:, HD:2 * HD], k_sl, k_sl,
                                start=False, stop=False, perf_mode=DR)
                            nc.tensor.matmul(
                                cg_ps[:, 2 * HD:3 * HD], q_sl, k_sl,
                                start=False, stop=(i == np_ - 1),
                                perf_mode=DR)
                        nc.vector.tensor_add(
                            cg_accum[:, h, :], cg_ps, cg_accum[:, h, :])
                    state["qk_pair"] = []

            def vtstage(t, xT_bf):
                # vT = w_vT.T @ xT_bf (feature-major, straight into the
                # c-major vt128 — the fused projection consumes it as-is)
                t0v = t * TOK_TILE
                for m in range(KK):
                    vps = ps_mm.tile([128, TOK_TILE], FP32, name="vps",
                                     tag="s")
                    for kk in range(KK):
                        nc.tensor.matmul(
                            vps, w_vT[:, kk, m * 128:(m + 1) * 128],
                            xT_bf[:, kk, :],
                            start=(kk == 0), stop=(kk == KK - 1))
                    if m % 2 == 0:
                        nc.scalar.mul(
                            vt128[:, m, t0v:t0v + TOK_TILE], vps, 1.0 / S_W)
                    else:
                        nc.vector.tensor_scalar_mul(
                            vt128[:, m, t0v:t0v + TOK_TILE], vps, 1.0 / S_W)

            # w prep in groups of 4 row-blocks: one wide psum + one cast per
            # (group, kk) instead of 4 narrow ones, using the ps_mm banks
            # that sit idle until the first qk matmul. Interleaved with the
            # first two x stages: the first qk matmul only needs w_qk
            # p-block 0 (m 0..3) + xT(0).
            xT01 = [None, None]
            with tc.tile_pool(name="wload", bufs=2) as wload:
                def wprep(grp):
                    w_blk = wload.tile([128, 2, C], BF16, name="w_blk")
                    nc.gpsimd.dma_start(
                        w_blk,
                        wqkv_d[grp * 256:(grp + 1) * 256, :].rearrange(
                            "(b p) f -> p b f", p=128))
                    for kk in range(KK):
                        tps = ps_mm.tile([128, 256], BF16, name="wps",
                                         tag="s")
                        for b in range(2):
                            nc.tensor.transpose(
                                tps[:, b * 128:(b + 1) * 128],
                                w_blk[:, b, kk * 128:(kk + 1) * 128], ident)
                        if grp < 6:
                            dst = w_qk_f8[grp // 2][
                                :, kk, (grp % 2) * 256:(grp % 2) * 256 + 256]
                            if (grp + kk) % 2 == 0:
                                nc.vector.tensor_scalar_mul(dst, tps, S_W)
                            else:
                                nc.scalar.mul(dst, tps, S_W)
                        else:
                            base = (grp - 6) * 256
                            dst = w_vT[:, kk, base:base + 256]
                            if kk % 2 == 0:
                                nc.vector.tensor_scalar_mul(dst, tps, S_W)
                            else:
                                nc.scalar.mul(dst, tps, S_W)

                xT01[0] = xstage(0)
                wprep(0)
                wprep(1)
                xT01[1] = xstage(1)
                wprep(2)
                wprep(3)
                for grp in range(4, 9):
                    wprep(grp)

            for t in range(N_TILES):
                xts = xT01[t] if t < 2 else state.pop(("x", t))
                mmstage(t, *xts, inline_vt=(t < 6))
                if t >= 6:
                    state[("xts", t)] = xts
                if t + 2 < N_TILES:
                    state[("x", t + 2)] = xstage(t + 2)

            phase2(nc, tc, singles, dram, ps_tr, cg_accum, attn_bf, temp_all,
                   ident96f)

            # deferred vT for the last tile pair: fills the PE while the
            # phase-2 DVE/ACT softmax chain runs
            vtstage(6, state.pop(("xts", 6))[0])
            vtstage(7, state.pop(("xts", 7))[0])

        # ---------------- phase 3 pools; w_projT96 prep + the B = attn.T
        # @ w_proj.T fold emitted first: they fill the PE while the DVE/ACT
        # softmax chain runs, and delete attn@v + both strip relayouts ----
        with tc.tile_pool(name="wpp", bufs=1) as wpp, \
             tc.tile_pool(name="wpload", bufs=2) as wpload, \
             tc.tile_pool(name="yp", bufs=2) as yp, \
             tc.tile_pool(name="ps_b", bufs=2, space="PSUM") as ps_b, \
             tc.tile_pool(name="ps_y", bufs=4, space="PSUM") as ps_y:
            # w_proj (cout, c) -> w_projT96 [96, h, cout] (per-head c rows)
            w_projT = wpp.tile([HD, H, C], BF16)
            for n in range(KK):
                wp_blk = wpload.tile([128, C], BF16, name="wp_blk")
                nc.gpsimd.dma_start(wp_blk, wproj_d[n * 128:(n + 1) * 128, :])
                for h in range(H):
                    tps2 = ps_tr.tile([HD, 128], BF16, name="tps2", tag="tr")
                    nc.tensor.transpose(
                        tps2, wp_blk[:, h * HD:(h + 1) * HD], ident)
                    if (n + h) % 2 == 0:
                        nc.vector.tensor_copy(
                            w_projT[:, h, n * 128:(n + 1) * 128], tps2)
                    else:
                        nc.scalar.copy(
                            w_projT[:, h, n * 128:(n + 1) * 128], tps2)

            # B[(h,e), co] = sum_d attn_h[d,e] * wp[co, 96h+d]:
            # per head one [96, 768] result in two psum pieces
            b96 = wpp.tile([HD, H, C], BF16)
            b128 = wpp.tile([128, KK, C], BF16)
            for (off, width) in ((0, 512), (512, 256)):
                for h in range(H):
                    bps = ps_b.tile([HD, 512], FP32, name="bps")
                    nc.tensor.matmul(
                        bps[:, :width], attn_bf[:, h, :],
                        w_projT[:, h, off:off + width],
                        start=True, stop=True)
                    if h % 2 == 0:
                        nc.vector.tensor_copy(
                            b96[:, h, off:off + width], bps[:, :width])
                    else:
                        nc.scalar.copy(
                            b96[:, h, off:off + width], bps[:, :width])
                # relayout rows c=96h+e -> dense 128-blocks, per column half
                # so the first projection psums start before B is complete
                for si, (m, p0, run, h, d0) in enumerate(_vt_strips()):
                    src = b96[d0:d0 + run, h, off:off + width]
                    dst = b128[p0:p0 + run, m, off:off + width]
                    if si % 3 == 0:
                        nc.gpsimd.dma_start(dst, src)
                    elif si % 3 == 1:
                        nc.sync.dma_start(dst, src)
                    else:
                        nc.scalar.dma_start(dst, src)

            # ---------------- phase 3: y = vt128.T @ b128 + b ------------
            for t in range(N_TILES):
                for piece in range(2):
                    t0 = t * TOK_TILE + piece * 256
                    y_t = yp.tile([128, 2, C], FP32, name="y_t")
                    for c in range(2):
                        cc = t * CHUNKS + piece * 2 + c
                        for (off, width) in ((0, 512), (512, 256)):
                            yps = ps_y.tile([128, 512], FP32, name="yps")
                            for m in range(KK):
                                nc.tensor.matmul(
                                    yps[:, :width],
                                    vt128[:, m, cc * 128:(cc + 1) * 128],
                                    b128[:, m, off:off + width],
                                    start=(m == 0), stop=(m == KK - 1))
                            nc.vector.tensor_tensor(
                                y_t[:, c, off:off + width], yps[:, :width],
                                b_all[:, off:off + width], mybir.AluOpType.add)
                    nc.scalar.dma_start(
                        out_d[t0:t0 + 256, :].rearrange(
                            "(c p) f -> p c f", p=128),
                        y_t)


def _get_nc():
    global _CACHED_NC
    if _CACHED_NC is None:
        _CACHED_NC = build_nc()
    return _CACHED_NC


def kernel(x, w_qkv, temperature, w_proj, b_proj):
    nc = _get_nc()
    x = np.ascontiguousarray(np.asarray(x, dtype=np.float32))
    in_maps = []
    for b in range(8):
        in_maps.append({
            "x": x[b],
            "w_qkv": np.asarray(w_qkv, dtype=np.float32),
            "temperature": np.asarray(temperature, dtype=np.float32),
            "w_proj": np.asarray(w_proj, dtype=np.float32),
            "b_proj": np.asarray(b_proj, dtype=np.float32),
        })
    res = run_bass_kernel_spmd(nc, in_maps, core_ids=list(range(8)))
    return np.stack([r["out"] for r in res.results], axis=0)
